# revision 28
# baseline (speedup 1.0000x reference)
"""Multi-head causal self-attention (B=2, S=2048, D=1024, H=16) on 8 NeuronCores.

Sharding: core c handles batch b = c // 4 and heads 4j..4j+3 where j = c % 4
(tensor-parallel over heads within a 4-core group, data-parallel over batch).

Structure (v3):
  * K/Q projections run in fp8e4 DoubleRow mode (x and Wq/Wk host-converted
    to fp8; 2 k-subtiles per matmul).  The fp8 quantization error only
    perturbs attention scores, which the 1/sqrt(S) softmax scale makes
    negligible.  V / out projections and S / PV matmuls stay bf16.
  * attention blocks are interleaved across the two head pairs by q-block:
    (0,qb) then (1,qb).  After both pairs of a q-block finish, the
    normalized ctx^T for all 4 local heads AllGathers across the 4-core
    group ([4*DH, QB] -> [D, QB]) and the out-projection for those queries
    is injected into later blocks.  Gathered feature order == natural head
    order, so Wo needs no row permutation.
  * V s-tiles are emitted just-in-time inside the pair-0 block that first
    needs them (before the corresponding PV), so attention starts ~4us in.
  * scores for diagonal k-tiles skip the fully-masked columns.
  * DMA batching: one ctxg load per gather, packed [128,QB] cn stores, and
    4-tile-wide output stores to relieve the HWDGE dispatch queue.

Per-head softmax denominator comes from an appended ones-column in V (row DH
of the ctx PSUM tile).  Heads are processed in pairs sharing 128 partitions
(rows 0-63 = even head, 64-127 = odd head of the pair).
"""

import math

import numpy as np
import ml_dtypes

import concourse.tile as tile
from concourse import bacc, mybir
from concourse.bass_utils import run_bass_kernel_spmd

B, S, D, H, DH = 2, 2048, 1024, 16, 64
NCORES = 8
GROUP = 4          # cores per batch group
HPC = 4            # heads per core
FPC = HPC * DH     # 256 features per core
QB = 512           # q block width
KT = 128           # k tile height (partition dim)
SCALE = 1.0 / math.sqrt(S)

F32 = mybir.dt.float32
BF16 = mybir.dt.bfloat16
FP8 = mybir.dt.float8e4
EXP = mybir.ActivationFunctionType.Exp
BF = ml_dtypes.bfloat16
F8 = ml_dtypes.float8_e4m3
DR = mybir.MatmulPerfMode.DoubleRow


def build_program(sim_collective=False, reps=1):
    """sim_collective=True replaces the AllGathers with equivalent-volume
    local DMA traffic so the (single-core) TimelineSim cost model can run.
    reps>1 repeats the whole body inside one NEFF (for slope timing)."""
    nc = bacc.Bacc(
        "TRN2",
        target_bir_lowering=False,
        debug=False,
        num_devices=NCORES,
    )

    xT = nc.dram_tensor("xT", [D, S], BF16, kind="ExternalInput").ap()
    x8 = nc.dram_tensor("x8", [D, S], FP8, kind="ExternalInput").ap()
    wq = nc.dram_tensor("wq", [D, FPC], FP8, kind="ExternalInput").ap()
    wk = nc.dram_tensor("wk", [D, FPC], FP8, kind="ExternalInput").ap()
    wv = nc.dram_tensor("wv", [D, FPC], BF16, kind="ExternalInput").ap()
    wo = nc.dram_tensor("wo", [D, FPC], BF16, kind="ExternalInput").ap()
    bo = nc.dram_tensor("bo", [1, FPC], F32, kind="ExternalInput").ap()
    tri = nc.dram_tensor("tri", [KT, 2 * KT], BF16, kind="ExternalInput").ap()
    ones = nc.dram_tensor("ones", [128, 16 * HPC], BF16, kind="ExternalInput").ap()
    out = nc.dram_tensor("out", [S, FPC], F32, kind="ExternalOutput").ap()

    with tile.TileContext(nc) as tc:
      for _rep in range(reps):
        with (
            tc.tile_pool(name="cpool", bufs=1) as cpool,
            tc.tile_pool(name="qkvp", bufs=1) as qkvp,
            tc.tile_pool(name="dpool", bufs=1, space="DRAM") as dpool,
        ):
            # ---- persistent SBUF tensors ---------------------------------
            wq_sb = cpool.tile([128, 8, FPC], FP8)
            wk_sb = cpool.tile([128, 8, FPC], FP8)
            wv_sb = cpool.tile([128, 8, FPC], BF16)
            wo_sb = cpool.tile([128, 8, FPC], BF16)
            xt_sb = cpool.tile([128, 8, S], BF16)
            x8_sb = cpool.tile([128, 8, S], FP8)
            tri_sb = cpool.tile([KT, 2, KT], BF16)
            bias_bc = cpool.tile([128, FPC], F32)

            qT_sb = qkvp.tile([128, 2, S], BF16)   # [dh-pair, pair, seq]
            kT_sb = qkvp.tile([128, 2, S], BF16)
            v_sb = qkvp.tile([128, 16, HPC * (DH + 1)], BF16)
            v4 = v_sb.rearrange("p s (h e) -> p s h e", e=DH + 1)

            cc_in = [dpool.tile([FPC, QB], BF16, name=f"cc_in{i}")
                     for i in range(3)]
            cc_out = [dpool.tile([GROUP * FPC, QB], BF16, name=f"cc_out{i}")
                      for i in range(3)]
            # qb3 gathers in two half-width pieces to pipeline the tail
            cc3_in = [dpool.tile([FPC, QB // 2], BF16, name=f"cc3_in{i}")
                      for i in range(2)]
            cc3_out = [dpool.tile([GROUP * FPC, QB // 2], BF16,
                                  name=f"cc3_out{i}") for i in range(2)]

            # ---- DMA loads, in consumption order -------------------------
            # K/Q critical path (fp8) on the SP queue; V path (bf16 xt, wv)
            # on the Act HWDGE queue so startup loads run in parallel.
            wq_d = wq.rearrange("(t p) f -> p t f", p=128)
            wk_d = wk.rearrange("(t p) f -> p t f", p=128)
            x8_d = x8.rearrange("(t p) m -> p t m", p=128)
            xt_dram_a = xT.rearrange("(t p) m -> p t m", p=128)
            nc.sync.dma_start(wk_sb[:], wk_d)
            nc.sync.dma_start(x8_sb[:, 0:4, 0:QB], x8_d[:, 0:4, 0:QB])
            nc.sync.dma_start(x8_sb[:, 4:8, 0:QB], x8_d[:, 4:8, 0:QB])
            nc.sync.dma_start(wq_sb[:], wq_d)
            nc.sync.dma_start(tri_sb[:], tri.rearrange("p (h q) -> p h q", q=KT))
            nc.sync.dma_start(
                v4[:, :, :, DH], ones.rearrange("p (s h) -> p s h", h=HPC)
            )
            wv_d = wv.rearrange("(t p) f -> p t f", p=128)
            nc.sync.dma_start(xt_sb[:, :, 0:128], xt_dram_a[:, :, 0:128])
            nc.sync.dma_start(wv_sb[:, 0:4], wv_d[:, 0:4])
            nc.sync.dma_start(xt_sb[:, :, 128:256], xt_dram_a[:, :, 128:256])
            nc.sync.dma_start(wv_sb[:, 4:8], wv_d[:, 4:8])
            nc.sync.dma_start(xt_sb[:, :, 256:QB], xt_dram_a[:, :, 256:QB])
            for c in range(1, 4):
                cs = slice(c * QB, (c + 1) * QB)
                nc.sync.dma_start(x8_sb[:, :, cs], x8_d[:, :, cs])
                nc.sync.dma_start(xt_sb[:, :, cs], xt_dram_a[:, :, cs])
            bo_sb = cpool.tile([1, FPC], F32)
            nc.sync.dma_start(bo_sb[:], bo)
            nc.gpsimd.partition_broadcast(bias_bc[:], bo_sb[:])
            nc.sync.dma_start(wo_sb[:], wo.rearrange("(t p) f -> p t f", p=128))

            # ---- pools ----------------------------------------------------
            # PSUM (8 banks): pj 2x[128,512]f32 (2, right; also holds V-proj
            # and out-proj tiles) + st 2x[128,2,512]f32 (4) + ctx 2x[65,512]
            # f32 (2).
            attps = tc.alloc_tile_pool(name="attps", bufs=1, space="PSUM")
            pjp = tc.alloc_tile_pool(name="pjp", bufs=1, space="PSUM",
                                     side="right")
            attp = tc.alloc_tile_pool(name="attp", bufs=8)
            nrmp = tc.alloc_tile_pool(name="nrmp", bufs=4)
            ogp = tc.alloc_tile_pool(name="ogp", bufs=1)
            obp = tc.alloc_tile_pool(name="obp", bufs=2)

            ctxg = ogp.tile([128, 8, S], BF16, name="ctxg", tag="ctxg")
            ots = {}
            op_ps = {}

            def emit_v(s):
                ps = pjp.tile([128, FPC], F32, tag="pj", bufs=2,
                              name=f"pv_{s}")
                for t in range(8):
                    nc.tensor.matmul(
                        ps[:],
                        xt_sb[:, t, s * 128:(s + 1) * 128],
                        wv_sb[:, t],
                        start=(t == 0),
                        stop=(t == 7),
                    )
                nc.vector.tensor_copy(
                    v4[:, s, :, 0:DH],
                    ps.rearrange("p (h e) -> p h e", e=DH),
                )

            KQ_FP8 = True

            def emit_kq(f, w_sb, dst, qb):
                ps = pjp.tile([128, QB], F32, tag="pj", bufs=2,
                              name=f"pkq_{f}_{qb}_{0 if w_sb is wk_sb else 1}")
                if KQ_FP8:
                    for t in range(4):
                        nc.tensor.matmul(
                            ps[:],
                            w_sb[:, 2 * t:2 * t + 2, f * 128:(f + 1) * 128],
                            x8_sb[:, 2 * t:2 * t + 2, qb * QB:(qb + 1) * QB],
                            start=(t == 0),
                            stop=(t == 3),
                            perf_mode=DR,
                        )
                else:
                    for t in range(8):
                        nc.tensor.matmul(
                            ps[:],
                            w_sb[:, t, f * 128:(f + 1) * 128],
                            x8_sb[:, t, qb * QB:(qb + 1) * QB],
                            start=(t == 0),
                            stop=(t == 7),
                        )
                nc.vector.tensor_copy(dst[:, f, qb * QB:(qb + 1) * QB], ps[:])

            K, Q = 0, 1

            def kq(f, which, qb):
                w, d = (wk_sb, kT_sb) if which == K else (wq_sb, qT_sb)
                return lambda: emit_kq(f, w, d, qb)

            # out-proj tile: 128 q rows x this core's 256 out columns;
            # emitted in two half-units (4 matmuls each) for fine-grained
            # injection; output staged in groups of 4 s-tiles for one
            # batched store.
            ops = {}

            def emit_op_half(s, half):
                g, i = divmod(s, 4)
                if half == 0:
                    op_ps[s] = pjp.tile([128, FPC], F32, tag="pj", bufs=2,
                                        name=f"op_{s}")
                ps = op_ps[s]
                for f in range(4 * half, 4 * half + 4):
                    nc.tensor.matmul(
                        ps[:],
                        ctxg[:, f, s * 128:(s + 1) * 128],
                        wo_sb[:, f],
                        start=(f == 0),
                        stop=(f == 7),
                    )
                if half == 1:
                    if i == 0:
                        ots[g] = obp.tile([128, 4, FPC], F32, tag="ot",
                                          name=f"ot_{g}")
                    nc.vector.tensor_add(ots[g][:, i], ps[:], bias_bc[:])
                    if i == 3:
                        nc.sync.dma_start(
                            out.rearrange("(g t p) f -> g p t f",
                                          g=4, p=128)[g],
                            ots[g][:],
                        )

            def emit_op(s):
                emit_op_half(s, 0)
                emit_op_half(s, 1)

            def op(s):
                return [lambda s=s: emit_op_half(s, 0),
                        lambda s=s: emit_op_half(s, 1)]

            def _gather(cin, cout, col0, ncol):
                if sim_collective:
                    for g in range(GROUP):
                        nc.sync.dma_start(
                            cout[g * FPC:(g + 1) * FPC, :], cin[:],
                        )
                else:
                    nc.gpsimd.collective_compute(
                        "AllGather",
                        mybir.AluOpType.bypass,
                        replica_groups=[[0, 1, 2, 3], [4, 5, 6, 7]],
                        ins=[cin.opt()],
                        outs=[cout.opt()],
                    )
                nc.sync.dma_start(
                    ctxg[:, :, col0:col0 + ncol],
                    cout.rearrange("(f p) q -> p f q", p=128),
                )

            def gather(qb):
                _gather(cc_in[qb], cc_out[qb], qb * QB, QB)

            def gather3(i):
                _gather(cc3_in[i], cc3_out[i],
                        3 * QB + i * (QB // 2), QB // 2)

            def attention_block(pair, qb, inject=(), new_v=False):
                inject = list(inject)
                h0, h1 = 2 * pair, 2 * pair + 1
                nk = 4 * (qb + 1)
                q0 = qb * QB
                ctx0 = attps.tile([DH + 1, QB], F32, tag="ctx", bufs=2,
                                  name=f"ctx0_{pair}_{qb}")
                ctx1 = attps.tile([DH + 1, QB], F32, tag="ctx", bufs=2,
                                  name=f"ctx1_{pair}_{qb}")
                sts = [None] * nk
                pts = [None] * nk

                def emit_s(ki):
                    ks = slice(ki * KT, (ki + 1) * KT)
                    off = max(ki * KT - q0, 0)
                    qs = slice(q0 + off, q0 + QB)
                    st = attps.tile([128, 2, QB], F32, tag="st", bufs=2,
                                    name=f"st_{pair}_{qb}_{ki}")
                    nc.tensor.matmul(
                        st[:, 0, off:], kT_sb[0:64, pair, ks],
                        qT_sb[0:64, pair, qs], start=True, stop=True,
                    )
                    nc.tensor.matmul(
                        st[:, 1, off:], kT_sb[64:128, pair, ks],
                        qT_sb[64:128, pair, qs], start=True, stop=True,
                    )
                    sts[ki] = st

                def emit_exp(ki):
                    off = max(ki * KT - q0, 0)
                    pt = attp.tile([128, 2, QB], BF16, tag="pt",
                                   name=f"pt_{pair}_{qb}_{ki}")
                    nc.scalar.activation(
                        pt[:, :, off:], sts[ki][:, :, off:], EXP, scale=SCALE,
                    )
                    if ki * KT - q0 >= 0:
                        nc.vector.tensor_mul(
                            pt[:, :, off:off + KT],
                            pt[:, :, off:off + KT],
                            tri_sb[:],
                        )
                    pts[ki] = pt

                def emit_pv(ki):
                    pt = pts[ki]
                    off = max(ki * KT - q0, 0)
                    nc.tensor.matmul(
                        ctx0[:, off:], v4[:, ki, h0], pt[:, 0, off:],
                        start=(ki == 0), stop=(ki == nk - 1),
                    )
                    nc.tensor.matmul(
                        ctx1[:, off:], v4[:, ki, h1], pt[:, 1, off:],
                        start=(ki == 0), stop=(ki == nk - 1),
                    )

                emit_s(0)
                emit_exp(0)
                for ki in range(nk):
                    if ki + 1 < nk:
                        emit_s(ki + 1)
                    if new_v and ki >= 4 * qb:
                        emit_v(ki)
                    # filler fires between S(ki+1) and PV(ki): the PE chews
                    # it exactly while waiting for exp(ki) to finish
                    if ki >= 1 and inject:
                        fn = inject.pop(0)
                        if fn is not None:
                            fn()
                    emit_pv(ki)
                    if ki + 1 < nk:
                        emit_exp(ki + 1)
                for fn in inject:
                    if fn is not None:
                        fn()

                # normalize ctx^T (bf16, both heads packed) and store to the
                # gather input rows for this pair
                rc0 = nrmp.tile([1, QB], F32, tag="rc0", name=f"rc0_{pair}_{qb}")
                nc.vector.reciprocal(rc0[:], ctx0[DH:DH + 1, :])
                rc1 = nrmp.tile([1, QB], F32, tag="rc1", name=f"rc1_{pair}_{qb}")
                nc.vector.reciprocal(rc1[:], ctx1[DH:DH + 1, :])
                # two partition-0-based bc tiles: gpsimd broadcast to a
                # partition-offset destination is unreliable on hardware
                bc0 = nrmp.tile([64, QB], F32, tag="bc0", name=f"bc0_{pair}_{qb}")
                nc.gpsimd.partition_broadcast(bc0[:], rc0[:])
                bc1 = nrmp.tile([64, QB], F32, tag="bc1", name=f"bc1_{pair}_{qb}")
                nc.gpsimd.partition_broadcast(bc1[:], rc1[:])
                cn = nrmp.tile([128, QB], BF16, tag="cn", name=f"cn_{pair}_{qb}")
                nc.vector.tensor_mul(cn[0:DH], ctx0[0:DH, :], bc0[:])
                nc.vector.tensor_mul(cn[DH:2 * DH], ctx1[0:DH, :], bc1[:])
                rows = slice(pair * 2 * DH, (pair + 1) * 2 * DH)
                if qb < 3:
                    nc.sync.dma_start(cc_in[qb][rows, :], cn[:])
                else:
                    hw = QB // 2
                    nc.sync.dma_start(cc3_in[0][rows, :], cn[:, 0:hw])
                    nc.sync.dma_start(cc3_in[1][rows, :], cn[:, hw:])

            # ---- era 1: first K/Q projections ----------------------------
            emit_kq(0, wk_sb, kT_sb, 0)
            emit_kq(0, wq_sb, qT_sb, 0)

            # ---- interleaved attention: (0,qb) then (1,qb), gather per qb;
            # V tiles emitted just-in-time in pair-0 blocks, projections one
            # block ahead, out-proj injected once its gather has landed.
            attention_block(0, 0, [kq(1, K, 0), kq(1, Q, 0)], new_v=True)
            attention_block(1, 0, [kq(0, K, 1), kq(0, Q, 1)])
            gather(0)
            attention_block(0, 1, [kq(1, K, 1), kq(1, Q, 1)], new_v=True)
            attention_block(1, 1, [kq(0, K, 2), kq(0, Q, 2), None]
                            + op(0) + op(1))
            gather(1)
            attention_block(0, 2, [kq(1, K, 2), kq(1, Q, 2), None]
                            + op(2) + op(3), new_v=True)
            attention_block(1, 2, [kq(0, K, 3), kq(0, Q, 3)]
                            + op(4) + op(5) + op(6))
            gather(2)
            attention_block(0, 3, [kq(1, K, 3), kq(1, Q, 3)]
                            + op(7) + op(8) + op(9), new_v=True)
            attention_block(1, 3, op(10) + op(11))
            gather3(0)
            gather3(1)
            for s in range(12, 16):
                emit_op(s)

            obp.release()
            ogp.release()
            nrmp.release()
            attp.release()
            attps.release()
            pjp.release()

    nc.compile()
    return nc


_PROGRAM = None


def _get_program():
    global _PROGRAM
    if _PROGRAM is None:
        _PROGRAM = build_program()
    return _PROGRAM


def _make_tri():
    # tri[i, j] = 1 where key-offset i <= query-offset j (allowed); two
    # copies along the free dim serve the two heads of a fused pair tile
    i = np.arange(KT)[:, None]
    j = np.arange(KT)[None, :]
    t = (i <= j).astype(np.float32)
    return np.concatenate([t, t], axis=1)


def make_in_maps(x, Wq, Wk, Wv, Wo, bo):
    tri_arr = _make_tri().astype(BF)
    ones_arr = np.ones((128, 16 * HPC), BF)
    xTs = [np.ascontiguousarray(x[b].T).astype(BF) for b in range(B)]
    xTs8 = [np.ascontiguousarray(x[b].T).astype(F8) for b in range(B)]
    in_maps = []
    for c in range(NCORES):
        b, j = divmod(c, GROUP)
        cols = slice(FPC * j, FPC * (j + 1))
        in_maps.append({
            "xT": xTs[b],
            "x8": xTs8[b],
            "wq": np.ascontiguousarray(Wq[:, cols]).astype(F8),
            "wk": np.ascontiguousarray(Wk[:, cols]).astype(F8),
            "wv": np.ascontiguousarray(Wv[:, cols]).astype(BF),
            "wo": np.ascontiguousarray(Wo[:, cols]).astype(BF),
            "bo": np.ascontiguousarray(bo[cols][None, :]).astype(np.float32),
            "tri": tri_arr,
            "ones": ones_arr,
        })
    return in_maps


def kernel(x, Wq, Wk, Wv, Wo, bo):
    x = np.ascontiguousarray(np.asarray(x, np.float32))
    Wq = np.asarray(Wq, np.float32)
    Wk = np.asarray(Wk, np.float32)
    Wv = np.asarray(Wv, np.float32)
    Wo = np.asarray(Wo, np.float32)
    bo = np.asarray(bo, np.float32)

    in_maps = make_in_maps(x, Wq, Wk, Wv, Wo, bo)
    nc = _get_program()
    results = run_bass_kernel_spmd(nc, in_maps, list(range(NCORES))).results

    out = np.empty((B, S, D), np.float32)
    for c in range(NCORES):
        b, j = divmod(c, GROUP)
        out[b, :, FPC * j:FPC * (j + 1)] = np.asarray(results[c]["out"],
                                                      np.float32)
    return out


# revision 33
# speedup vs baseline: 1.0917x; 1.0917x over previous
"""Multi-head causal self-attention (B=2, S=2048, D=1024, H=16) on 8 NeuronCores.

Sharding: core c handles batch b = c // 4 and heads 4j..4j+3 where j = c % 4
(tensor-parallel over heads within a 4-core group, data-parallel over batch).

Structure (v3):
  * K/Q projections run in fp8e4 DoubleRow mode (x and Wq/Wk host-converted
    to fp8; 2 k-subtiles per matmul).  The fp8 quantization error only
    perturbs attention scores, which the 1/sqrt(S) softmax scale makes
    negligible.  V / out projections and S / PV matmuls stay bf16.
  * attention blocks are interleaved across the two head pairs by q-block:
    (0,qb) then (1,qb).  After both pairs of a q-block finish, the
    normalized ctx^T for all 4 local heads AllGathers across the 4-core
    group ([4*DH, QB] -> [D, QB]) and the out-projection for those queries
    is injected into later blocks.  Gathered feature order == natural head
    order, so Wo needs no row permutation.
  * V s-tiles are emitted just-in-time inside the pair-0 block that first
    needs them (before the corresponding PV), so attention starts ~4us in.
  * scores for diagonal k-tiles skip the fully-masked columns.
  * DMA batching: one ctxg load per gather, packed [128,QB] cn stores, and
    4-tile-wide output stores to relieve the HWDGE dispatch queue.

Per-head softmax denominator comes from an appended ones-column in V (row DH
of the ctx PSUM tile).  Heads are processed in pairs sharing 128 partitions
(rows 0-63 = even head, 64-127 = odd head of the pair).
"""

import math

import numpy as np
import ml_dtypes

import concourse.tile as tile
from concourse import bacc, mybir
from concourse.bass_utils import run_bass_kernel_spmd

B, S, D, H, DH = 2, 2048, 1024, 16, 64
NCORES = 8
GROUP = 4          # cores per batch group
HPC = 4            # heads per core
FPC = HPC * DH     # 256 features per core
QB = 512           # q block width
KT = 128           # k tile height (partition dim)
SCALE = 1.0 / math.sqrt(S)

F32 = mybir.dt.float32
BF16 = mybir.dt.bfloat16
FP8 = mybir.dt.float8e4
EXP = mybir.ActivationFunctionType.Exp
BF = ml_dtypes.bfloat16
F8 = ml_dtypes.float8_e4m3
DR = mybir.MatmulPerfMode.DoubleRow


def build_program(sim_collective=False, reps=1):
    """sim_collective=True replaces the AllGathers with equivalent-volume
    local DMA traffic so the (single-core) TimelineSim cost model can run.
    reps>1 repeats the whole body inside one NEFF (for slope timing)."""
    nc = bacc.Bacc(
        "TRN2",
        target_bir_lowering=False,
        debug=False,
        num_devices=NCORES,
    )

    xT = nc.dram_tensor("xT", [D, S], BF16, kind="ExternalInput").ap()
    x8 = nc.dram_tensor("x8", [D, S], FP8, kind="ExternalInput").ap()
    wq = nc.dram_tensor("wq", [D, FPC], FP8, kind="ExternalInput").ap()
    wk = nc.dram_tensor("wk", [D, FPC], FP8, kind="ExternalInput").ap()
    wv = nc.dram_tensor("wv", [D, FPC], BF16, kind="ExternalInput").ap()
    wo = nc.dram_tensor("wo", [D, FPC], BF16, kind="ExternalInput").ap()
    bo = nc.dram_tensor("bo", [1, FPC], F32, kind="ExternalInput").ap()
    tri = nc.dram_tensor("tri", [KT, 2 * KT], BF16, kind="ExternalInput").ap()
    ones = nc.dram_tensor("ones", [128, 16 * HPC], BF16, kind="ExternalInput").ap()
    out = nc.dram_tensor("out", [S, FPC], F32, kind="ExternalOutput").ap()

    with tile.TileContext(nc) as tc:
      for _rep in range(reps):
        with (
            tc.tile_pool(name="cpool", bufs=1) as cpool,
            tc.tile_pool(name="qkvp", bufs=1) as qkvp,
            tc.tile_pool(name="dpool", bufs=1, space="DRAM") as dpool,
        ):
            # ---- persistent SBUF tensors ---------------------------------
            wq_sb = cpool.tile([128, 8, FPC], FP8)
            wk_sb = cpool.tile([128, 8, FPC], FP8)
            wv_sb = cpool.tile([128, 8, FPC], BF16)
            wo_sb = cpool.tile([128, 8, FPC], BF16)
            xt_sb = cpool.tile([128, 8, S], BF16)
            x8_sb = cpool.tile([128, 8, S], FP8)
            tri_sb = cpool.tile([KT, 2, KT], BF16)
            bias_bc = cpool.tile([128, FPC], F32)

            qT_sb = qkvp.tile([128, 2, S], BF16)   # [dh-pair, pair, seq]
            kT_sb = qkvp.tile([128, 2, S], BF16)
            v_sb = qkvp.tile([128, 16, HPC * (DH + 1)], BF16)
            v4 = v_sb.rearrange("p s (h e) -> p s h e", e=DH + 1)

            # v1-style comm: 3 AllGathers — pair-0 full after its 4 blocks,
            # pair-1 q<1536, pair-1 tail.  Collectives have ~15us fixed
            # latency, so fewer, earlier-emitted gathers beat per-qb ones.
            cc_in0 = dpool.tile([2 * DH, S], BF16)
            cc_in1a = dpool.tile([2 * DH, 3 * S // 4], BF16)
            cc_in1b = dpool.tile([2 * DH, S // 4], BF16)
            cc_out0 = dpool.tile([GROUP * 2 * DH, S], BF16)
            cc_out1a = dpool.tile([GROUP * 2 * DH, 3 * S // 4], BF16)
            cc_out1b = dpool.tile([GROUP * 2 * DH, S // 4], BF16)

            # ---- DMA loads, in consumption order -------------------------
            # K/Q critical path (fp8) on the SP queue; V path (bf16 xt, wv)
            # on the Act HWDGE queue so startup loads run in parallel.
            wq_d = wq.rearrange("(t p) f -> p t f", p=128)
            wk_d = wk.rearrange("(t p) f -> p t f", p=128)
            x8_d = x8.rearrange("(t p) m -> p t m", p=128)
            xt_dram_a = xT.rearrange("(t p) m -> p t m", p=128)
            nc.sync.dma_start(wk_sb[:], wk_d)
            nc.sync.dma_start(x8_sb[:, 0:4, 0:QB], x8_d[:, 0:4, 0:QB])
            nc.sync.dma_start(x8_sb[:, 4:8, 0:QB], x8_d[:, 4:8, 0:QB])
            nc.sync.dma_start(wq_sb[:], wq_d)
            nc.sync.dma_start(tri_sb[:], tri.rearrange("p (h q) -> p h q", q=KT))
            nc.sync.dma_start(
                v4[:, :, :, DH], ones.rearrange("p (s h) -> p s h", h=HPC)
            )
            wv_d = wv.rearrange("(t p) f -> p t f", p=128)
            nc.sync.dma_start(xt_sb[:, :, 0:128], xt_dram_a[:, :, 0:128])
            nc.sync.dma_start(wv_sb[:, 0:4], wv_d[:, 0:4])
            nc.sync.dma_start(xt_sb[:, :, 128:256], xt_dram_a[:, :, 128:256])
            nc.sync.dma_start(wv_sb[:, 4:8], wv_d[:, 4:8])
            nc.sync.dma_start(xt_sb[:, :, 256:QB], xt_dram_a[:, :, 256:QB])
            for c in range(1, 4):
                cs = slice(c * QB, (c + 1) * QB)
                nc.sync.dma_start(x8_sb[:, :, cs], x8_d[:, :, cs])
                nc.sync.dma_start(xt_sb[:, :, cs], xt_dram_a[:, :, cs])
            bo_sb = cpool.tile([1, FPC], F32)
            nc.sync.dma_start(bo_sb[:], bo)
            nc.gpsimd.partition_broadcast(bias_bc[:], bo_sb[:])
            nc.sync.dma_start(wo_sb[:], wo.rearrange("(t p) f -> p t f", p=128))

            # ---- pools ----------------------------------------------------
            # PSUM (8 banks): pj 2x[128,512]f32 (2, right; also holds V-proj
            # and out-proj tiles) + st 2x[128,2,512]f32 (4) + ctx 2x[65,512]
            # f32 (2).
            attps = tc.alloc_tile_pool(name="attps", bufs=1, space="PSUM")
            pjp = tc.alloc_tile_pool(name="pjp", bufs=1, space="PSUM",
                                     side="right")
            attp = tc.alloc_tile_pool(name="attp", bufs=8)
            nrmp = tc.alloc_tile_pool(name="nrmp", bufs=4)
            ogp = tc.alloc_tile_pool(name="ogp", bufs=1)
            obp = tc.alloc_tile_pool(name="obp", bufs=2)

            ctxg = ogp.tile([128, 8, S], BF16, name="ctxg", tag="ctxg")
            ots = {}
            op_ps = {}

            def emit_v(s):
                ps = pjp.tile([128, FPC], F32, tag="pj", bufs=2,
                              name=f"pv_{s}")
                for t in range(8):
                    nc.tensor.matmul(
                        ps[:],
                        xt_sb[:, t, s * 128:(s + 1) * 128],
                        wv_sb[:, t],
                        start=(t == 0),
                        stop=(t == 7),
                    )
                nc.vector.tensor_copy(
                    v4[:, s, :, 0:DH],
                    ps.rearrange("p (h e) -> p h e", e=DH),
                )

            KQ_FP8 = True

            def emit_kq(f, w_sb, dst, qb):
                ps = pjp.tile([128, QB], F32, tag="pj", bufs=2,
                              name=f"pkq_{f}_{qb}_{0 if w_sb is wk_sb else 1}")
                if KQ_FP8:
                    for t in range(4):
                        nc.tensor.matmul(
                            ps[:],
                            w_sb[:, 2 * t:2 * t + 2, f * 128:(f + 1) * 128],
                            x8_sb[:, 2 * t:2 * t + 2, qb * QB:(qb + 1) * QB],
                            start=(t == 0),
                            stop=(t == 3),
                            perf_mode=DR,
                        )
                else:
                    for t in range(8):
                        nc.tensor.matmul(
                            ps[:],
                            w_sb[:, t, f * 128:(f + 1) * 128],
                            x8_sb[:, t, qb * QB:(qb + 1) * QB],
                            start=(t == 0),
                            stop=(t == 7),
                        )
                nc.vector.tensor_copy(dst[:, f, qb * QB:(qb + 1) * QB], ps[:])

            K, Q = 0, 1

            def kq(f, which, qb):
                w, d = (wk_sb, kT_sb) if which == K else (wq_sb, qT_sb)
                return lambda: emit_kq(f, w, d, qb)

            # out-proj tile: 128 q rows x this core's 256 out columns;
            # emitted in two half-units (4 matmuls each) for fine-grained
            # injection; output staged in groups of 4 s-tiles for one
            # batched store.
            ops = {}

            def emit_op_half(s, half):
                g, i = divmod(s, 4)
                if half == 0:
                    op_ps[s] = pjp.tile([128, FPC], F32, tag="pj", bufs=2,
                                        name=f"op_{s}")
                ps = op_ps[s]
                for f in range(4 * half, 4 * half + 4):
                    nc.tensor.matmul(
                        ps[:],
                        ctxg[:, f, s * 128:(s + 1) * 128],
                        wo_sb[:, f],
                        start=(f == 0),
                        stop=(f == 7),
                    )
                if half == 1:
                    if i == 0:
                        ots[g] = obp.tile([128, 4, FPC], F32, tag="ot",
                                          name=f"ot_{g}")
                    nc.vector.tensor_add(ots[g][:, i], ps[:], bias_bc[:])
                    if i == 3:
                        nc.sync.dma_start(
                            out.rearrange("(g t p) f -> g p t f",
                                          g=4, p=128)[g],
                            ots[g][:],
                        )

            def emit_op(s):
                emit_op_half(s, 0)
                emit_op_half(s, 1)

            def op(s):
                return [lambda s=s: emit_op_half(s, 0),
                        lambda s=s: emit_op_half(s, 1)]

            def _gather(cin, cout, f0, col0, ncol):
                if sim_collective:
                    for g in range(GROUP):
                        nc.sync.dma_start(
                            cout[g * 2 * DH:(g + 1) * 2 * DH, :], cin[:],
                        )
                else:
                    nc.gpsimd.collective_compute(
                        "AllGather",
                        mybir.AluOpType.bypass,
                        replica_groups=[[0, 1, 2, 3], [4, 5, 6, 7]],
                        ins=[cin.opt()],
                        outs=[cout.opt()],
                    )
                nc.sync.dma_start(
                    ctxg[:, f0:f0 + 4, col0:col0 + ncol],
                    cout.rearrange("(f p) q -> p f q", p=128),
                )

            def attention_block(pair, qb, inject=(), new_v=False):
                inject = list(inject)
                h0, h1 = 2 * pair, 2 * pair + 1
                nk = 4 * (qb + 1)
                q0 = qb * QB
                ctx0 = attps.tile([DH + 1, QB], F32, tag="ctx", bufs=2,
                                  name=f"ctx0_{pair}_{qb}")
                ctx1 = attps.tile([DH + 1, QB], F32, tag="ctx", bufs=2,
                                  name=f"ctx1_{pair}_{qb}")
                sts = [None] * nk
                pts = [None] * nk

                def emit_s(ki):
                    ks = slice(ki * KT, (ki + 1) * KT)
                    off = max(ki * KT - q0, 0)
                    qs = slice(q0 + off, q0 + QB)
                    st = attps.tile([128, 2, QB], F32, tag="st", bufs=2,
                                    name=f"st_{pair}_{qb}_{ki}")
                    nc.tensor.matmul(
                        st[:, 0, off:], kT_sb[0:64, pair, ks],
                        qT_sb[0:64, pair, qs], start=True, stop=True,
                    )
                    nc.tensor.matmul(
                        st[:, 1, off:], kT_sb[64:128, pair, ks],
                        qT_sb[64:128, pair, qs], start=True, stop=True,
                    )
                    sts[ki] = st

                def emit_exp(ki):
                    off = max(ki * KT - q0, 0)
                    pt = attp.tile([128, 2, QB], BF16, tag="pt",
                                   name=f"pt_{pair}_{qb}_{ki}")
                    nc.scalar.activation(
                        pt[:, :, off:], sts[ki][:, :, off:], EXP, scale=SCALE,
                    )
                    if ki * KT - q0 >= 0:
                        nc.vector.tensor_mul(
                            pt[:, :, off:off + KT],
                            pt[:, :, off:off + KT],
                            tri_sb[:],
                        )
                    pts[ki] = pt

                def emit_pv(ki):
                    pt = pts[ki]
                    off = max(ki * KT - q0, 0)
                    nc.tensor.matmul(
                        ctx0[:, off:], v4[:, ki, h0], pt[:, 0, off:],
                        start=(ki == 0), stop=(ki == nk - 1),
                    )
                    nc.tensor.matmul(
                        ctx1[:, off:], v4[:, ki, h1], pt[:, 1, off:],
                        start=(ki == 0), stop=(ki == nk - 1),
                    )

                emit_s(0)
                emit_exp(0)
                for ki in range(nk):
                    if ki + 1 < nk:
                        emit_s(ki + 1)
                    if new_v and ki >= 4 * qb:
                        emit_v(ki)
                    # filler fires between S(ki+1) and PV(ki): the PE chews
                    # it exactly while waiting for exp(ki) to finish
                    if ki >= 1 and inject:
                        fn = inject.pop(0)
                        if fn is not None:
                            fn()
                    emit_pv(ki)
                    if ki + 1 < nk:
                        emit_exp(ki + 1)
                for fn in inject:
                    if fn is not None:
                        fn()

                # normalize ctx^T (bf16, both heads packed) and store to the
                # gather input rows for this pair
                rc0 = nrmp.tile([1, QB], F32, tag="rc0", name=f"rc0_{pair}_{qb}")
                nc.vector.reciprocal(rc0[:], ctx0[DH:DH + 1, :])
                rc1 = nrmp.tile([1, QB], F32, tag="rc1", name=f"rc1_{pair}_{qb}")
                nc.vector.reciprocal(rc1[:], ctx1[DH:DH + 1, :])
                # two partition-0-based bc tiles: gpsimd broadcast to a
                # partition-offset destination is unreliable on hardware
                bc0 = nrmp.tile([64, QB], F32, tag="bc0", name=f"bc0_{pair}_{qb}")
                nc.gpsimd.partition_broadcast(bc0[:], rc0[:])
                bc1 = nrmp.tile([64, QB], F32, tag="bc1", name=f"bc1_{pair}_{qb}")
                nc.gpsimd.partition_broadcast(bc1[:], rc1[:])
                cn = nrmp.tile([128, QB], BF16, tag="cn", name=f"cn_{pair}_{qb}")
                nc.vector.tensor_mul(cn[0:DH], ctx0[0:DH, :], bc0[:])
                nc.vector.tensor_mul(cn[DH:2 * DH], ctx1[0:DH, :], bc1[:])
                qs = slice(q0, q0 + QB)
                if pair == 0:
                    nc.sync.dma_start(cc_in0[:, qs], cn[:])
                elif qb < 3:
                    nc.sync.dma_start(cc_in1a[:, qs], cn[:])
                else:
                    nc.sync.dma_start(cc_in1b[:, :], cn[:])

            # ---- era 1: first K/Q projections ----------------------------
            emit_kq(0, wk_sb, kT_sb, 0)
            emit_kq(0, wq_sb, qT_sb, 0)

            # ---- pair-0 blocks (V just-in-time, K/Q one block ahead) -----
            attention_block(0, 0, [kq(0, K, 1), kq(0, Q, 1)], new_v=True)
            attention_block(0, 1, [kq(0, K, 2), kq(0, Q, 2)], new_v=True)
            attention_block(0, 2, [kq(0, K, 3), kq(0, Q, 3)], new_v=True)
            attention_block(0, 3, [kq(1, K, 0), kq(1, Q, 0),
                                   kq(1, K, 1), kq(1, Q, 1)], new_v=True)
            # pair-0 ctx complete: gather it (f-blocks 0-3 of ctxg)
            _gather(cc_in0, cc_out0, 0, 0, S)

            # ---- pair-1 blocks, out-projection injected into the last ----
            attention_block(1, 0, [kq(1, K, 2), kq(1, Q, 2)])
            attention_block(1, 1, [kq(1, K, 3), kq(1, Q, 3)])
            attention_block(1, 2)
            # pair-1 q<1536 gathered; f-blocks 4-7 for those columns
            _gather(cc_in1a, cc_out1a, 4, 0, 3 * S // 4)
            attention_block(1, 3, [(lambda s=s: emit_op(s))
                                   for s in range(12)])
            _gather(cc_in1b, cc_out1b, 4, 3 * S // 4, S // 4)
            for s in range(12, 16):
                emit_op(s)

            obp.release()
            ogp.release()
            nrmp.release()
            attp.release()
            attps.release()
            pjp.release()

    nc.compile()
    return nc


_PROGRAM = None


def _get_program():
    global _PROGRAM
    if _PROGRAM is None:
        _PROGRAM = build_program()
    return _PROGRAM


def _make_tri():
    # tri[i, j] = 1 where key-offset i <= query-offset j (allowed); two
    # copies along the free dim serve the two heads of a fused pair tile
    i = np.arange(KT)[:, None]
    j = np.arange(KT)[None, :]
    t = (i <= j).astype(np.float32)
    return np.concatenate([t, t], axis=1)


def make_in_maps(x, Wq, Wk, Wv, Wo, bo):
    tri_arr = _make_tri().astype(BF)
    ones_arr = np.ones((128, 16 * HPC), BF)
    xTs = [np.ascontiguousarray(x[b].T).astype(BF) for b in range(B)]
    xTs8 = [np.ascontiguousarray(x[b].T).astype(F8) for b in range(B)]
    # Wo rows permuted to match the gathered ctx^T feature order:
    # gather0 rows = (rank j, heads 4j+0, 4j+1), gather1 = (rank j, 4j+2, 4j+3)
    perm = [4 * j + p for g in range(2) for j in range(GROUP)
            for p in (2 * g, 2 * g + 1)]
    Wo_perm = Wo.reshape(H, DH, D)[perm].reshape(D, D)
    in_maps = []
    for c in range(NCORES):
        b, j = divmod(c, GROUP)
        cols = slice(FPC * j, FPC * (j + 1))
        in_maps.append({
            "xT": xTs[b],
            "x8": xTs8[b],
            "wq": np.ascontiguousarray(Wq[:, cols]).astype(F8),
            "wk": np.ascontiguousarray(Wk[:, cols]).astype(F8),
            "wv": np.ascontiguousarray(Wv[:, cols]).astype(BF),
            "wo": np.ascontiguousarray(Wo_perm[:, cols]).astype(BF),
            "bo": np.ascontiguousarray(bo[cols][None, :]).astype(np.float32),
            "tri": tri_arr,
            "ones": ones_arr,
        })
    return in_maps


def kernel(x, Wq, Wk, Wv, Wo, bo):
    x = np.ascontiguousarray(np.asarray(x, np.float32))
    Wq = np.asarray(Wq, np.float32)
    Wk = np.asarray(Wk, np.float32)
    Wv = np.asarray(Wv, np.float32)
    Wo = np.asarray(Wo, np.float32)
    bo = np.asarray(bo, np.float32)

    in_maps = make_in_maps(x, Wq, Wk, Wv, Wo, bo)
    nc = _get_program()
    results = run_bass_kernel_spmd(nc, in_maps, list(range(NCORES))).results

    out = np.empty((B, S, D), np.float32)
    for c in range(NCORES):
        b, j = divmod(c, GROUP)
        out[b, :, FPC * j:FPC * (j + 1)] = np.asarray(results[c]["out"],
                                                      np.float32)
    return out


# revision 38
# speedup vs baseline: 1.1445x; 1.0484x over previous
"""Multi-head causal self-attention (B=2, S=2048, D=1024, H=16) on 8 NeuronCores.

Sharding: core c handles batch b = c // 4 and heads 4j..4j+3 where j = c % 4
(tensor-parallel over heads within a 4-core group, data-parallel over batch).

Structure (vs the 180us baseline):
  * K/Q projections run in fp8e4 DoubleRow mode (x and Wq/Wk host-converted
    to fp8; 2 k-subtiles per matmul, half the instructions and 2-4x the
    per-instruction throughput).  The fp8 quantization error only perturbs
    attention scores, which the 1/sqrt(S) softmax scale makes negligible
    (overall rel err 4.55e-3 vs 4.73e-3 all-bf16).  V / out projections
    and S / PV matmuls stay bf16 for accuracy.
  * V s-tiles are emitted just-in-time inside the pair-0 block that first
    needs them (right before the corresponding PV), and the first K/Q
    block's loads are split fine-grained, so the attention pipeline (and
    the Act-engine exp stream) starts ~5us in instead of ~21us.
  * injected filler (remaining projections / out-proj) fires between
    S(ki+1) and PV(ki), exactly where the PE would stall waiting for exp.
  * comm stays in the baseline's proven 3-AllGather shape (collectives
    carry ~15us fixed latency, so per-q-block gathers lose): pair-0 ctx^T
    full after its 4 blocks, pair-1 q<1536, pair-1 tail; each gather's
    ctxg load is a single batched DMA into one [128, 8, S] tile.
  * scores for diagonal k-tiles skip the fully-masked columns.
  * normalized ctx^T of both heads is packed into one [128, QB] tile
    (one store per block), and output tiles are staged 4-wide per store
    to relieve the ~625ns-per-dispatch HWDGE queue.

Per-head softmax denominator comes from an appended ones-column in V (row DH
of the ctx PSUM tile).  Heads are processed in pairs sharing 128 partitions
(rows 0-63 = even head, 64-127 = odd head of the pair).
"""

import math

import numpy as np
import ml_dtypes

import concourse.tile as tile
from concourse import bacc, mybir
from concourse.bass_utils import run_bass_kernel_spmd

B, S, D, H, DH = 2, 2048, 1024, 16, 64
NCORES = 8
GROUP = 4          # cores per batch group
HPC = 4            # heads per core
FPC = HPC * DH     # 256 features per core
QB = 512           # q block width
KT = 128           # k tile height (partition dim)
SCALE = 1.0 / math.sqrt(S)

F32 = mybir.dt.float32
BF16 = mybir.dt.bfloat16
FP8 = mybir.dt.float8e4
EXP = mybir.ActivationFunctionType.Exp
BF = ml_dtypes.bfloat16
F8 = ml_dtypes.float8_e4m3
DR = mybir.MatmulPerfMode.DoubleRow


def build_program(sim_collective=False, reps=1):
    """sim_collective=True replaces the AllGathers with equivalent-volume
    local DMA traffic so the (single-core) TimelineSim cost model can run.
    reps>1 repeats the whole body inside one NEFF (for slope timing)."""
    nc = bacc.Bacc(
        "TRN2",
        target_bir_lowering=False,
        debug=False,
        num_devices=NCORES,
    )

    xT = nc.dram_tensor("xT", [D, S], BF16, kind="ExternalInput").ap()
    x8 = nc.dram_tensor("x8", [D, S], FP8, kind="ExternalInput").ap()
    wq = nc.dram_tensor("wq", [D, FPC], FP8, kind="ExternalInput").ap()
    wk = nc.dram_tensor("wk", [D, FPC], FP8, kind="ExternalInput").ap()
    wv = nc.dram_tensor("wv", [D, FPC], BF16, kind="ExternalInput").ap()
    wo = nc.dram_tensor("wo", [D, FPC], BF16, kind="ExternalInput").ap()
    bo = nc.dram_tensor("bo", [1, FPC], F32, kind="ExternalInput").ap()
    tri = nc.dram_tensor("tri", [KT, 2 * KT], BF16, kind="ExternalInput").ap()
    ones = nc.dram_tensor("ones", [128, 16 * HPC], BF16, kind="ExternalInput").ap()
    out = nc.dram_tensor("out", [S, FPC], F32, kind="ExternalOutput").ap()

    with tile.TileContext(nc) as tc:
      for _rep in range(reps):
        with (
            tc.tile_pool(name="cpool", bufs=1) as cpool,
            tc.tile_pool(name="qkvp", bufs=1) as qkvp,
            tc.tile_pool(name="dpool", bufs=1, space="DRAM") as dpool,
        ):
            # ---- persistent SBUF tensors ---------------------------------
            wq_sb = cpool.tile([128, 8, FPC], FP8)
            wk_sb = cpool.tile([128, 8, FPC], FP8)
            wv_sb = cpool.tile([128, 8, FPC], BF16)
            wo_sb = cpool.tile([128, 8, FPC], BF16)
            xt_sb = cpool.tile([128, 8, S], BF16)
            x8_sb = cpool.tile([128, 8, S], FP8)
            tri_sb = cpool.tile([KT, 2, KT], BF16)
            bias_bc = cpool.tile([128, FPC], F32)

            qT_sb = qkvp.tile([128, 2, S], BF16)   # [dh-pair, pair, seq]
            kT_sb = qkvp.tile([128, 2, S], BF16)
            v_sb = qkvp.tile([128, 16, HPC * (DH + 1)], BF16)
            v4 = v_sb.rearrange("p s (h e) -> p s h e", e=DH + 1)

            # v1-style comm: 3 AllGathers — pair-0 full after its 4 blocks,
            # pair-1 q<1536, pair-1 tail.  Collectives have ~15us fixed
            # latency, so fewer, earlier-emitted gathers beat per-qb ones.
            cc_in0 = dpool.tile([2 * DH, S], BF16)
            cc_in1a = dpool.tile([2 * DH, S // 2], BF16)
            cc_in1b = dpool.tile([2 * DH, S // 4], BF16)
            cc_in1c = dpool.tile([2 * DH, S // 4], BF16)
            cc_out0 = dpool.tile([GROUP * 2 * DH, S], BF16)
            cc_out1a = dpool.tile([GROUP * 2 * DH, S // 2], BF16)
            cc_out1b = dpool.tile([GROUP * 2 * DH, S // 4], BF16)
            cc_out1c = dpool.tile([GROUP * 2 * DH, S // 4], BF16)

            # ---- DMA loads, in consumption order -------------------------
            # K/Q critical path (fp8) on the SP queue; V path (bf16 xt, wv)
            # on the Act HWDGE queue so startup loads run in parallel.
            wq_d = wq.rearrange("(t p) f -> p t f", p=128)
            wk_d = wk.rearrange("(t p) f -> p t f", p=128)
            x8_d = x8.rearrange("(t p) m -> p t m", p=128)
            xt_dram_a = xT.rearrange("(t p) m -> p t m", p=128)
            nc.sync.dma_start(wk_sb[:], wk_d)
            nc.sync.dma_start(x8_sb[:, 0:4, 0:QB], x8_d[:, 0:4, 0:QB])
            nc.sync.dma_start(x8_sb[:, 4:8, 0:QB], x8_d[:, 4:8, 0:QB])
            nc.sync.dma_start(wq_sb[:], wq_d)
            nc.sync.dma_start(tri_sb[:], tri.rearrange("p (h q) -> p h q", q=KT))
            nc.sync.dma_start(
                v4[:, :, :, DH], ones.rearrange("p (s h) -> p s h", h=HPC)
            )
            wv_d = wv.rearrange("(t p) f -> p t f", p=128)
            nc.sync.dma_start(xt_sb[:, :, 0:128], xt_dram_a[:, :, 0:128])
            nc.sync.dma_start(wv_sb[:, 0:4], wv_d[:, 0:4])
            nc.sync.dma_start(xt_sb[:, :, 128:256], xt_dram_a[:, :, 128:256])
            nc.sync.dma_start(wv_sb[:, 4:8], wv_d[:, 4:8])
            nc.sync.dma_start(xt_sb[:, :, 256:QB], xt_dram_a[:, :, 256:QB])
            for c in range(1, 4):
                cs = slice(c * QB, (c + 1) * QB)
                nc.sync.dma_start(x8_sb[:, :, cs], x8_d[:, :, cs])
                nc.sync.dma_start(xt_sb[:, :, cs], xt_dram_a[:, :, cs])
            bo_sb = cpool.tile([1, FPC], F32)
            nc.sync.dma_start(bo_sb[:], bo)
            nc.gpsimd.partition_broadcast(bias_bc[:], bo_sb[:])
            nc.sync.dma_start(wo_sb[:], wo.rearrange("(t p) f -> p t f", p=128))

            # ---- pools ----------------------------------------------------
            # PSUM (8 banks): pj 2x[128,512]f32 (2, right; also holds V-proj
            # and out-proj tiles) + st 2x[128,2,512]f32 (4) + ctx 2x[65,512]
            # f32 (2).
            attps = tc.alloc_tile_pool(name="attps", bufs=1, space="PSUM")
            pjp = tc.alloc_tile_pool(name="pjp", bufs=1, space="PSUM",
                                     side="right")
            attp = tc.alloc_tile_pool(name="attp", bufs=8)
            nrmp = tc.alloc_tile_pool(name="nrmp", bufs=4)
            ogp = tc.alloc_tile_pool(name="ogp", bufs=1)
            obp = tc.alloc_tile_pool(name="obp", bufs=2)

            ctxg = ogp.tile([128, 8, S], BF16, name="ctxg", tag="ctxg")
            ots = {}
            op_ps = {}

            def emit_v(s):
                ps = pjp.tile([128, FPC], F32, tag="pj", bufs=2,
                              name=f"pv_{s}")
                for t in range(8):
                    nc.tensor.matmul(
                        ps[:],
                        xt_sb[:, t, s * 128:(s + 1) * 128],
                        wv_sb[:, t],
                        start=(t == 0),
                        stop=(t == 7),
                    )
                nc.vector.tensor_copy(
                    v4[:, s, :, 0:DH],
                    ps.rearrange("p (h e) -> p h e", e=DH),
                )

            KQ_FP8 = True

            def emit_kq(f, w_sb, dst, qb):
                ps = pjp.tile([128, QB], F32, tag="pj", bufs=2,
                              name=f"pkq_{f}_{qb}_{0 if w_sb is wk_sb else 1}")
                if KQ_FP8:
                    for t in range(4):
                        nc.tensor.matmul(
                            ps[:],
                            w_sb[:, 2 * t:2 * t + 2, f * 128:(f + 1) * 128],
                            x8_sb[:, 2 * t:2 * t + 2, qb * QB:(qb + 1) * QB],
                            start=(t == 0),
                            stop=(t == 3),
                            perf_mode=DR,
                        )
                else:
                    for t in range(8):
                        nc.tensor.matmul(
                            ps[:],
                            w_sb[:, t, f * 128:(f + 1) * 128],
                            x8_sb[:, t, qb * QB:(qb + 1) * QB],
                            start=(t == 0),
                            stop=(t == 7),
                        )
                nc.vector.tensor_copy(dst[:, f, qb * QB:(qb + 1) * QB], ps[:])

            K, Q = 0, 1

            def kq(f, which, qb):
                w, d = (wk_sb, kT_sb) if which == K else (wq_sb, qT_sb)
                return lambda: emit_kq(f, w, d, qb)

            # out-proj tile: 128 q rows x this core's 256 out columns;
            # emitted in two half-units (4 matmuls each) for fine-grained
            # injection; output staged in groups of 4 s-tiles for one
            # batched store.
            ops = {}

            def emit_op_half(s, half):
                g, i = divmod(s, 4)
                if half == 0:
                    op_ps[s] = pjp.tile([128, FPC], F32, tag="pj", bufs=2,
                                        name=f"op_{s}")
                ps = op_ps[s]
                for f in range(4 * half, 4 * half + 4):
                    nc.tensor.matmul(
                        ps[:],
                        ctxg[:, f, s * 128:(s + 1) * 128],
                        wo_sb[:, f],
                        start=(f == 0),
                        stop=(f == 7),
                    )
                if half == 1:
                    if i == 0:
                        ots[g] = obp.tile([128, 4, FPC], F32, tag="ot",
                                          name=f"ot_{g}")
                    nc.vector.tensor_add(ots[g][:, i], ps[:], bias_bc[:])
                    if i == 3:
                        nc.sync.dma_start(
                            out.rearrange("(g t p) f -> g p t f",
                                          g=4, p=128)[g],
                            ots[g][:],
                        )

            def emit_op(s):
                emit_op_half(s, 0)
                emit_op_half(s, 1)

            def op(s):
                return [lambda s=s: emit_op_half(s, 0),
                        lambda s=s: emit_op_half(s, 1)]

            def _gather(cin, cout, f0, col0, ncol):
                if sim_collective:
                    for g in range(GROUP):
                        nc.sync.dma_start(
                            cout[g * 2 * DH:(g + 1) * 2 * DH, :], cin[:],
                        )
                else:
                    nc.gpsimd.collective_compute(
                        "AllGather",
                        mybir.AluOpType.bypass,
                        replica_groups=[[0, 1, 2, 3], [4, 5, 6, 7]],
                        ins=[cin.opt()],
                        outs=[cout.opt()],
                    )
                nc.sync.dma_start(
                    ctxg[:, f0:f0 + 4, col0:col0 + ncol],
                    cout.rearrange("(f p) q -> p f q", p=128),
                )

            def attention_block(pair, qb, inject=(), new_v=False):
                inject = list(inject)
                h0, h1 = 2 * pair, 2 * pair + 1
                nk = 4 * (qb + 1)
                q0 = qb * QB
                ctx0 = attps.tile([DH + 1, QB], F32, tag="ctx", bufs=2,
                                  name=f"ctx0_{pair}_{qb}")
                ctx1 = attps.tile([DH + 1, QB], F32, tag="ctx", bufs=2,
                                  name=f"ctx1_{pair}_{qb}")
                sts = [None] * nk
                pts = [None] * nk

                def emit_s(ki):
                    ks = slice(ki * KT, (ki + 1) * KT)
                    off = max(ki * KT - q0, 0)
                    qs = slice(q0 + off, q0 + QB)
                    st = attps.tile([128, 2, QB], F32, tag="st", bufs=2,
                                    name=f"st_{pair}_{qb}_{ki}")
                    nc.tensor.matmul(
                        st[:, 0, off:], kT_sb[0:64, pair, ks],
                        qT_sb[0:64, pair, qs], start=True, stop=True,
                    )
                    nc.tensor.matmul(
                        st[:, 1, off:], kT_sb[64:128, pair, ks],
                        qT_sb[64:128, pair, qs], start=True, stop=True,
                    )
                    sts[ki] = st

                def emit_exp(ki):
                    off = max(ki * KT - q0, 0)
                    pt = attp.tile([128, 2, QB], BF16, tag="pt",
                                   name=f"pt_{pair}_{qb}_{ki}")
                    nc.scalar.activation(
                        pt[:, :, off:], sts[ki][:, :, off:], EXP, scale=SCALE,
                    )
                    if ki * KT - q0 >= 0:
                        nc.vector.tensor_mul(
                            pt[:, :, off:off + KT],
                            pt[:, :, off:off + KT],
                            tri_sb[:],
                        )
                    pts[ki] = pt

                def emit_pv(ki):
                    pt = pts[ki]
                    off = max(ki * KT - q0, 0)
                    nc.tensor.matmul(
                        ctx0[:, off:], v4[:, ki, h0], pt[:, 0, off:],
                        start=(ki == 0), stop=(ki == nk - 1),
                    )
                    nc.tensor.matmul(
                        ctx1[:, off:], v4[:, ki, h1], pt[:, 1, off:],
                        start=(ki == 0), stop=(ki == nk - 1),
                    )

                emit_s(0)
                emit_exp(0)
                for ki in range(nk):
                    if ki + 1 < nk:
                        emit_s(ki + 1)
                    if new_v and ki >= 4 * qb:
                        emit_v(ki)
                    # filler fires between S(ki+1) and PV(ki): the PE chews
                    # it exactly while waiting for exp(ki) to finish
                    if ki >= 1 and inject:
                        fn = inject.pop(0)
                        if fn is not None:
                            fn()
                    emit_pv(ki)
                    if ki + 1 < nk:
                        emit_exp(ki + 1)
                for fn in inject:
                    if fn is not None:
                        fn()

                # normalize ctx^T (bf16, both heads packed) and store to the
                # gather input rows for this pair
                rc0 = nrmp.tile([1, QB], F32, tag="rc0", name=f"rc0_{pair}_{qb}")
                nc.vector.reciprocal(rc0[:], ctx0[DH:DH + 1, :])
                rc1 = nrmp.tile([1, QB], F32, tag="rc1", name=f"rc1_{pair}_{qb}")
                nc.vector.reciprocal(rc1[:], ctx1[DH:DH + 1, :])
                # two partition-0-based bc tiles: gpsimd broadcast to a
                # partition-offset destination is unreliable on hardware
                bc0 = nrmp.tile([64, QB], F32, tag="bc0", name=f"bc0_{pair}_{qb}")
                nc.gpsimd.partition_broadcast(bc0[:], rc0[:])
                bc1 = nrmp.tile([64, QB], F32, tag="bc1", name=f"bc1_{pair}_{qb}")
                nc.gpsimd.partition_broadcast(bc1[:], rc1[:])
                cn = nrmp.tile([128, QB], BF16, tag="cn", name=f"cn_{pair}_{qb}")
                nc.vector.tensor_mul(cn[0:DH], ctx0[0:DH, :], bc0[:])
                nc.vector.tensor_mul(cn[DH:2 * DH], ctx1[0:DH, :], bc1[:])
                if pair == 0:
                    nc.sync.dma_start(cc_in0[:, q0:q0 + QB], cn[:])
                elif qb < 2:
                    nc.sync.dma_start(cc_in1a[:, q0:q0 + QB], cn[:])
                elif qb == 2:
                    nc.sync.dma_start(cc_in1b[:, :], cn[:])
                else:
                    nc.sync.dma_start(cc_in1c[:, :], cn[:])

            # ---- era 1: first K/Q projections ----------------------------
            emit_kq(0, wk_sb, kT_sb, 0)
            emit_kq(0, wq_sb, qT_sb, 0)

            # ---- pair-0 blocks (V just-in-time, K/Q one block ahead) -----
            attention_block(0, 0, [kq(0, K, 1), kq(0, Q, 1)], new_v=True)
            attention_block(0, 1, [kq(0, K, 2), kq(0, Q, 2)], new_v=True)
            attention_block(0, 2, [kq(0, K, 3), kq(0, Q, 3)], new_v=True)
            attention_block(0, 3, [kq(1, K, 0), kq(1, Q, 0),
                                   kq(1, K, 1), kq(1, Q, 1)], new_v=True)
            # pair-0 ctx complete: gather it (f-blocks 0-3 of ctxg)
            _gather(cc_in0, cc_out0, 0, 0, S)

            # ---- pair-1 blocks; its ctx^T gathers in three pieces so the
            # out-projection can start in block (1,2) instead of cramming
            # all 16 tiles into (1,3) and the tail ------------------------
            attention_block(1, 0, [kq(1, K, 2), kq(1, Q, 2)])
            attention_block(1, 1, [kq(1, K, 3), kq(1, Q, 3)])
            _gather(cc_in1a, cc_out1a, 4, 0, S // 2)
            attention_block(1, 2, [None, None, None]
                            + [(lambda s=s: emit_op(s)) for s in range(6)])
            _gather(cc_in1b, cc_out1b, 4, S // 2, S // 4)
            # ops 6-11 need pieces a/b; the pair-0 half (f-blocks 0-3) of
            # tail ops 12-13 only needs gather0, so it runs in-block too
            # (only 2 preloads: each holds an open accumulation on one of
            # the two pj PSUM slots until its half-1 lands in the tail)
            attention_block(1, 3, [(lambda s=s: emit_op(s))
                                   for s in range(6, 12)]
                            + [(lambda s=s: emit_op_half(s, 0))
                               for s in range(12, 14)])
            _gather(cc_in1c, cc_out1c, 4, 3 * S // 4, S // 4)
            emit_op_half(12, 1)
            emit_op_half(13, 1)
            emit_op(14)
            emit_op(15)

            obp.release()
            ogp.release()
            nrmp.release()
            attp.release()
            attps.release()
            pjp.release()

    nc.compile()
    return nc


_PROGRAM = None


def _get_program():
    global _PROGRAM
    if _PROGRAM is None:
        _PROGRAM = build_program()
    return _PROGRAM


def _make_tri():
    # tri[i, j] = 1 where key-offset i <= query-offset j (allowed); two
    # copies along the free dim serve the two heads of a fused pair tile
    i = np.arange(KT)[:, None]
    j = np.arange(KT)[None, :]
    t = (i <= j).astype(np.float32)
    return np.concatenate([t, t], axis=1)


def make_in_maps(x, Wq, Wk, Wv, Wo, bo):
    tri_arr = _make_tri().astype(BF)
    ones_arr = np.ones((128, 16 * HPC), BF)
    xTs = [np.ascontiguousarray(x[b].T).astype(BF) for b in range(B)]
    xTs8 = [np.ascontiguousarray(x[b].T).astype(F8) for b in range(B)]
    # Wo rows permuted to match the gathered ctx^T feature order:
    # gather0 rows = (rank j, heads 4j+0, 4j+1), gather1 = (rank j, 4j+2, 4j+3)
    perm = [4 * j + p for g in range(2) for j in range(GROUP)
            for p in (2 * g, 2 * g + 1)]
    Wo_perm = Wo.reshape(H, DH, D)[perm].reshape(D, D)
    in_maps = []
    for c in range(NCORES):
        b, j = divmod(c, GROUP)
        cols = slice(FPC * j, FPC * (j + 1))
        in_maps.append({
            "xT": xTs[b],
            "x8": xTs8[b],
            "wq": np.ascontiguousarray(Wq[:, cols]).astype(F8),
            "wk": np.ascontiguousarray(Wk[:, cols]).astype(F8),
            "wv": np.ascontiguousarray(Wv[:, cols]).astype(BF),
            "wo": np.ascontiguousarray(Wo_perm[:, cols]).astype(BF),
            "bo": np.ascontiguousarray(bo[cols][None, :]).astype(np.float32),
            "tri": tri_arr,
            "ones": ones_arr,
        })
    return in_maps


def kernel(x, Wq, Wk, Wv, Wo, bo):
    x = np.ascontiguousarray(np.asarray(x, np.float32))
    Wq = np.asarray(Wq, np.float32)
    Wk = np.asarray(Wk, np.float32)
    Wv = np.asarray(Wv, np.float32)
    Wo = np.asarray(Wo, np.float32)
    bo = np.asarray(bo, np.float32)

    in_maps = make_in_maps(x, Wq, Wk, Wv, Wo, bo)
    nc = _get_program()
    results = run_bass_kernel_spmd(nc, in_maps, list(range(NCORES))).results

    out = np.empty((B, S, D), np.float32)
    for c in range(NCORES):
        b, j = divmod(c, GROUP)
        out[b, :, FPC * j:FPC * (j + 1)] = np.asarray(results[c]["out"],
                                                      np.float32)
    return out
